# revision 41
# baseline (speedup 1.0000x reference)
"""Biquad IIR filter (direct-form-II-transposed) on 8 Trainium2 NeuronCores.

Strategy
--------
The biquad is stable (|poles| <= ~0.72 for the spec's coefficient
distribution), so its impulse response decays below bf16 resolution well
within 128 taps.  The sequential IIR scan becomes a 128-tap FIR
convolution computed as a block-Toeplitz matmul with blocks of M=128:

    yT[:, j] = A1 @ xT[:, j] + A2 @ xT[:, j-1]
    A1[i,k] = h[i-k]        (lower triangular, current block)
    A2[i,k] = h[128+i-k]    (strict upper triangular, previous block tail)

where xT[k, j] = x[j*128 + k].  The host pre-transposes x into xT (and
un-transposes y afterwards), so the device never transposes anything:
the tensor engine runs exactly two bf16 matmuls (1 cycle/row) per
512-block chunk, with the Toeplitz factors as stationary operands and
xT as the moving operand; the A2 term is the same matmul with the
moving operand shifted one block-column (a zero/carry column baked into
xT col 0 handles the row start).  All tensors are bf16 (inputs rounded
on host, outputs rounded on device and upcast on host), which halves
HBM traffic; per-partition DMA runs are 8KB contiguous so DMA moves at
full rate.

Pipeline (raw Bass, one standalone wait per dependency):

    SP  : per-row xT loads on its HWDGE queue, then stores for rows 4-7
          (they enqueue strictly after the loads, so loads keep the full
          DMA bandwidth while they gate compute)
    ACT : w load first (arms its HWDGE queue early - a queue's first DMA
          pays ~8.5us), evacuates even chunks (PSUM f32 -> SBUF bf16),
          and stores rows 0-3 so stores overlap the load phase
    PE  : per chunk, A1/A2 matmuls into a 4-deep PSUM bank rotation
    DVE : evacuates odd chunks

The DMA engines process descriptors serially at ~26.5GB/s each (~424GB/s
aggregate), so the kernel is DMA-bound end to end: ~9.5us fixed NEFF/
queue-arming head + ~17.3MB of traffic + store tail.

Sharding: data-parallel over the batch axis - 64 rows / 8 cores = 8
rows per core; filters are per-row so there is no cross-core traffic.
"""

import sys

import numpy as np

if "/opt/trn_rl_repo" not in sys.path:
    sys.path.insert(0, "/opt/trn_rl_repo")

import ml_dtypes

import concourse.bass as bass
import concourse.mybir as mybir
from concourse.bass_utils import run_bass_kernel_spmd

BATCH = 64
T = 524288
NCORES = 8
R = BATCH // NCORES  # rows per core
NH = 128  # FIR taps (impulse response length kept)
M = 128  # block length = matmul contraction dim
NBLK = T // M  # 4096 blocks per row
CHUNK = 512  # blocks per chunk = one fp32 PSUM bank
NCH = NBLK // CHUNK  # chunks per row
F32 = mybir.dt.float32
BF16 = mybir.dt.bfloat16
NPBF16 = ml_dtypes.bfloat16

_CACHED = {}


def _impulse_response(b: np.ndarray, a: np.ndarray, n: int) -> np.ndarray:
    """First n samples of the biquad impulse response, computed in f64."""
    nb = b.astype(np.float64)
    na = a.astype(np.float64)
    b0, b1, b2 = nb[:, 0], nb[:, 1], nb[:, 2]
    a1, a2 = na[:, 0], na[:, 1]
    rows = b.shape[0]
    h = np.zeros((rows, n), dtype=np.float64)
    z1 = np.zeros(rows, dtype=np.float64)
    z2 = np.zeros(rows, dtype=np.float64)
    for t in range(n):
        v0 = 1.0 if t == 0 else 0.0
        v1 = b0 * v0 + z1
        nz1 = b1 * v0 - a1 * v1 + z2
        nz2 = b2 * v0 - a2 * v1
        h[:, t] = v1
        z1, z2 = nz1, nz2
    return h


def _toeplitz_weights(h: np.ndarray) -> tuple[np.ndarray, np.ndarray]:
    """Build per-row stationary matmul operands W1T/W2T, each [rows,128,128].

    W1T[r, k, i] = h[r, i-k]      for i >= k   (A1 transposed)
    W2T[r, k, i] = h[r, 128+i-k]  for k >  i   (A2 transposed)
    """
    rows = h.shape[0]
    i = np.arange(M)[None, :]  # output sample within block
    k = np.arange(M)[:, None]  # input sample within block
    d1 = i - k
    w1 = np.zeros((rows, M, M), dtype=np.float64)
    mask1 = d1 >= 0
    w1[:, mask1] = h[:, d1[mask1]]
    d2 = M + i - k
    w2 = np.zeros((rows, M, M), dtype=np.float64)
    mask2 = d2 <= NH - 1
    w2[:, mask2] = h[:, d2[mask2]]
    return w1, w2


class _Waiter:
    """Emit a standalone wait_ge only when the target value increases."""

    def __init__(self, eng):
        self.eng = eng
        self.seen = {}

    def need(self, sem, val):
        if val <= 0:
            return
        if self.seen.get(sem.name, -1) >= val:
            return
        self.seen[sem.name] = val
        self.eng.wait_ge(sem, val)


def _build_bass(rows: int = R) -> bass.Bass:
    ntot = rows * NCH

    nc = bass.Bass(trn_type="TRN2")
    # xT with a leading zero/carry column: x_d[r, k, 1+j] = x[r, j*128+k]
    x_d = nc.declare_dram_parameter("x", [rows, M, NBLK + 1], BF16, isOutput=False)
    # w_d[k, a, r, i] = WaT[r, k, i]
    w_d = nc.declare_dram_parameter("w", [M, 2 * rows * M], BF16, isOutput=False)
    # yT: y_d[r, k, j] = y[r, j*128+k]
    y_d = nc.declare_dram_parameter("y", [rows, M, NBLK], BF16, isOutput=True)

    # --- SBUF tensors ---
    w_s = nc.alloc_sbuf_tensor("w_s", [M, 2, rows, M], BF16).ap()
    xrow = [
        nc.alloc_sbuf_tensor(f"xrow{r}", [M, NBLK + 1], BF16).ap()
        for r in range(rows)
    ]
    yrow = [
        nc.alloc_sbuf_tensor(f"yrow{r}", [M, NBLK], BF16).ap() for r in range(rows)
    ]

    # --- PSUM: 8-deep rotation of fp32 banks for the matmul accumulator,
    # so the PE runs up to 7 chunks (a full row) ahead of the ACT/DVE
    # evacuations and never stalls on their semaphore round-trips ---
    NPS = 8
    y_ps = [
        nc.alloc_psum_tensor(f"yps{i}", [M, CHUNK], F32).ap() for i in range(NPS)
    ]

    with (
        nc.Block() as block,
        nc.semaphore("s_w") as s_w,
        nc.semaphore("s_x0") as s_x0,
        nc.semaphore("s_x1") as s_x1,
        nc.semaphore("s_x2") as s_x2,
        nc.semaphore("s_x3") as s_x3,
        nc.semaphore("s_x4") as s_x4,
        nc.semaphore("s_x5") as s_x5,
        nc.semaphore("s_x6") as s_x6,
        nc.semaphore("s_x7") as s_x7,
        nc.semaphore("s_pe") as s_pe,
        nc.semaphore("s_ya") as s_ya,
        nc.semaphore("s_yd") as s_yd,
        nc.semaphore("s_st") as s_st,
    ):
        s_x = [s_x0, s_x1, s_x2, s_x3, s_x4, s_x5, s_x6, s_x7][:rows]

        @block.tensor
        def _(pe: bass.BassEngine):
            W = _Waiter(pe)
            W.need(s_w, 16)
            gch = 0
            for r in range(rows):
                W.need(s_x[r], 16)
                for ch in range(NCH):
                    pp = gch % NPS
                    # WAR: y_ps[pp] must be evacuated from chunk gch-NPS
                    if gch >= NPS:
                        if gch % 2 == 0:
                            W.need(s_ya, (gch - NPS) // 2 + 1)
                        else:
                            W.need(s_yd, (gch - NPS - 1) // 2 + 1)
                    nc.tensor.matmul(
                        y_ps[pp],
                        lhsT=w_s[:, 0, r],
                        rhs=xrow[r][:, 1 + ch * CHUNK : 1 + (ch + 1) * CHUNK],
                        start=True,
                        stop=False,
                    )
                    nc.tensor.matmul(
                        y_ps[pp],
                        lhsT=w_s[:, 1, r],
                        rhs=xrow[r][:, ch * CHUNK : ch * CHUNK + CHUNK],
                        start=False,
                        stop=True,
                    ).then_inc(s_pe, 1)
                    gch += 1

        @block.scalar
        def _(a: bass.BassEngine):
            # ACT: loads w on its HWDGE queue first (this also arms the queue
            # early - a queue's first DMA pays ~8.5us), then evacuates even
            # chunks (PSUM banks 0/2); stores for the first half of the rows
            # also go on this queue so they overlap the SP-queue loads
            W = _Waiter(a)
            a.dma_start(
                out=w_s, in_=w_d.rearrange("k (a r i) -> k a r i", a=2, r=rows)
            ).then_inc(s_w, 16)
            for gch in range(0, ntot, 2):
                r, ch = divmod(gch, NCH)
                W.need(s_pe, gch + 1)
                a.copy(
                    out=yrow[r][:, ch * CHUNK : (ch + 1) * CHUNK],
                    in_=y_ps[gch % NPS],
                ).then_inc(s_ya, 1)
                if ch == NCH - 2 and r < rows // 2:
                    # row r fully evacuated once DVE finishes chunk r*NCH+7
                    W.need(s_yd, 4 * r + 4)
                    a.dma_start(out=y_d[r], in_=yrow[r]).then_inc(s_st, 16)
                if ch == NCH - 2 and r == rows - 1:
                    # last row: ACT takes the second half so the final store
                    # drains on both queues in parallel
                    W.need(s_yd, 4 * r + 4)
                    HB = CHUNK * (NCH // 2)
                    a.dma_start(
                        out=y_d[r][:, HB:], in_=yrow[r][:, HB:]
                    ).then_inc(s_st, 16)

        @block.vector
        def _(v: bass.BassEngine):
            # DVE evacuates odd chunks (PSUM banks 1/3)
            W = _Waiter(v)
            for gch in range(1, ntot, 2):
                r, ch = divmod(gch, NCH)
                W.need(s_pe, gch + 1)
                v.tensor_copy(
                    out=yrow[r][:, ch * CHUNK : (ch + 1) * CHUNK],
                    in_=y_ps[gch % NPS],
                ).then_inc(s_yd, 1)

        @block.sync
        def _(sp: bass.BassEngine):
            W = _Waiter(sp)
            # loads via the SP HWDGE queue: descriptor generation happens in
            # hardware, so the 8 row loads stream back-to-back at full rate
            for r in range(rows):
                sp.dma_start(out=xrow[r], in_=x_d[r]).then_inc(s_x[r], 16)
            # stores for the second half of the rows share the SP queue:
            # they enqueue strictly after the loads; the last row is split
            # with ACT taking its second half
            HB = CHUNK * (NCH // 2)
            for r in range(rows // 2, rows):
                W.need(s_ya, 4 * r + 4)
                W.need(s_yd, 4 * r + 4)
                if r == rows - 1:
                    sp.dma_start(
                        out=y_d[r][:, :HB], in_=yrow[r][:, :HB]
                    ).then_inc(s_st, 16)
                else:
                    sp.dma_start(out=y_d[r], in_=yrow[r]).then_inc(s_st, 16)
            W.need(s_st, 16 * (rows + 1))

    return nc


def _get_nc() -> bass.Bass:
    if "nc" not in _CACHED:
        _CACHED["nc"] = _build_bass()
    return _CACHED["nc"]


def run(x, b, a, trace=False, **spmd_kwargs):
    """Shard inputs, run the Bass kernel on 8 cores, gather full output."""
    x = np.asarray(x, dtype=np.float32)
    b = np.asarray(b, dtype=np.float32)
    a = np.asarray(a, dtype=np.float32)
    assert x.shape == (BATCH, T), x.shape
    h = _impulse_response(b, a, NH)
    w1, w2 = _toeplitz_weights(h)  # [BATCH, M(k), M(i)] each, f64
    # w_host[b, k, a, i] -> flatten per-row later
    w = np.stack([w1, w2], axis=2)  # [BATCH, M(k), 2, M(i)]
    w = np.ascontiguousarray(w.transpose(0, 2, 1, 3))  # [BATCH, 2, k, i]

    # host-side transpose: xT[b, k, j] = x[b, j*128+k], with zero col 0
    xt = np.zeros((BATCH, M, NBLK + 1), dtype=NPBF16)
    xt[:, :, 1:] = (
        x.reshape(BATCH, NBLK, M).transpose(0, 2, 1).astype(NPBF16)
    )

    in_maps = []
    for c in range(NCORES):
        rs = slice(c * R, (c + 1) * R)
        # w_core[k, a, r, i] layout flattened to [M, 2*R*M]
        wc = np.ascontiguousarray(
            w[rs].transpose(2, 1, 0, 3).reshape(M, 2 * R * M).astype(NPBF16)
        )
        in_maps.append({"x": np.ascontiguousarray(xt[rs]), "w": wc})
    nc = _get_nc()
    out = run_bass_kernel_spmd(
        nc, in_maps, list(range(NCORES)), trace=trace, **spmd_kwargs
    )
    yt = np.concatenate(
        [np.asarray(out.results[c]["y"]) for c in range(NCORES)], axis=0
    )  # [BATCH, M, NBLK] bf16
    y = yt.transpose(0, 2, 1).reshape(BATCH, T).astype(np.float32)
    return y, out


def kernel(x, b, a):
    y, _ = run(x, b, a)
    return y


# revision 43
# speedup vs baseline: 1.1426x; 1.1426x over previous
"""Biquad IIR filter (direct-form-II-transposed) on 8 Trainium2 NeuronCores.

Strategy
--------
The biquad is stable (|poles| <= ~0.72 for the spec's coefficient
distribution), so its impulse response decays below bf16 resolution well
within 128 taps.  The sequential IIR scan becomes a 128-tap FIR
convolution computed as a block-Toeplitz matmul with blocks of M=128:

    yT[:, j] = A1 @ xT[:, j] + A2 @ xT[:, j-1]
    A1[i,k] = h[i-k]        (lower triangular, current block)
    A2[i,k] = h[128+i-k]    (strict upper triangular, previous block tail)

where xT[k, j] = x[j*128 + k].  The host pre-transposes x into xT (and
un-transposes y afterwards), so the device never transposes anything:
the tensor engine runs exactly two bf16 matmuls (1 cycle/row) per
512-block chunk, with the Toeplitz factors as stationary operands and
xT as the moving operand; the A2 term is the same matmul with the
moving operand shifted one block-column (a zero/carry column baked into
xT col 0 handles the row start).  All tensors are bf16 (inputs rounded
on host, outputs rounded on device and upcast on host), which halves
HBM traffic; per-partition DMA runs are 8KB contiguous so DMA moves at
full rate.

Pipeline (raw Bass, one standalone wait per dependency):

    SP  : per-row xT loads on its HWDGE queue, then stores for rows 4-7
          (they enqueue strictly after the loads, so loads keep the full
          DMA bandwidth while they gate compute)
    ACT : w load first (arms its HWDGE queue early - a queue's first DMA
          pays ~8.5us), evacuates even chunks (PSUM f32 -> SBUF bf16),
          and stores rows 0-3 so stores overlap the load phase
    PE  : per chunk, A1/A2 matmuls into a 4-deep PSUM bank rotation
    DVE : evacuates odd chunks

The DMA engines process descriptors serially at ~26.5GB/s each (~424GB/s
aggregate), so the kernel is DMA-bound end to end: ~9.5us fixed NEFF/
queue-arming head + ~17.3MB of traffic + store tail.

Sharding: data-parallel over the batch axis - 64 rows / 8 cores = 8
rows per core; filters are per-row so there is no cross-core traffic.
"""

import sys

import numpy as np

if "/opt/trn_rl_repo" not in sys.path:
    sys.path.insert(0, "/opt/trn_rl_repo")

import ml_dtypes

import concourse.bass as bass
import concourse.mybir as mybir
from concourse.bass_utils import run_bass_kernel_spmd

BATCH = 64
T = 524288
NCORES = 8
R = BATCH // NCORES  # rows per core
NH = 128  # FIR taps (impulse response length kept)
M = 128  # block length = matmul contraction dim
NBLK = T // M  # 4096 blocks per row
CHUNK = 512  # blocks per chunk = one fp32 PSUM bank
NCH = NBLK // CHUNK  # chunks per row
F32 = mybir.dt.float32
BF16 = mybir.dt.bfloat16
NPBF16 = ml_dtypes.bfloat16

_CACHED = {}


def _impulse_response(b: np.ndarray, a: np.ndarray, n: int) -> np.ndarray:
    """First n samples of the biquad impulse response, computed in f64."""
    nb = b.astype(np.float64)
    na = a.astype(np.float64)
    b0, b1, b2 = nb[:, 0], nb[:, 1], nb[:, 2]
    a1, a2 = na[:, 0], na[:, 1]
    rows = b.shape[0]
    h = np.zeros((rows, n), dtype=np.float64)
    z1 = np.zeros(rows, dtype=np.float64)
    z2 = np.zeros(rows, dtype=np.float64)
    for t in range(n):
        v0 = 1.0 if t == 0 else 0.0
        v1 = b0 * v0 + z1
        nz1 = b1 * v0 - a1 * v1 + z2
        nz2 = b2 * v0 - a2 * v1
        h[:, t] = v1
        z1, z2 = nz1, nz2
    return h


def _toeplitz_weights(h: np.ndarray) -> tuple[np.ndarray, np.ndarray]:
    """Build per-row stationary matmul operands W1T/W2T, each [rows,128,128].

    W1T[r, k, i] = h[r, i-k]      for i >= k   (A1 transposed)
    W2T[r, k, i] = h[r, 128+i-k]  for k >  i   (A2 transposed)
    """
    rows = h.shape[0]
    i = np.arange(M)[None, :]  # output sample within block
    k = np.arange(M)[:, None]  # input sample within block
    d1 = i - k
    w1 = np.zeros((rows, M, M), dtype=np.float64)
    mask1 = d1 >= 0
    w1[:, mask1] = h[:, d1[mask1]]
    d2 = M + i - k
    w2 = np.zeros((rows, M, M), dtype=np.float64)
    mask2 = d2 <= NH - 1
    w2[:, mask2] = h[:, d2[mask2]]
    return w1, w2


class _Waiter:
    """Emit a standalone wait_ge only when the target value increases."""

    def __init__(self, eng):
        self.eng = eng
        self.seen = {}

    def need(self, sem, val):
        if val <= 0:
            return
        if self.seen.get(sem.name, -1) >= val:
            return
        self.seen[sem.name] = val
        self.eng.wait_ge(sem, val)


def _build_bass(rows: int = R) -> bass.Bass:
    ntot = rows * NCH

    nc = bass.Bass(trn_type="TRN2")
    # xT with a leading zero/carry column: x_d[r, k, 1+j] = x[r, j*128+k]
    x_d = nc.declare_dram_parameter("x", [rows, M, NBLK + 1], BF16, isOutput=False)
    # w_d[k, a, r, i] = WaT[r, k, i]
    w_d = nc.declare_dram_parameter("w", [M, 2 * rows * M], BF16, isOutput=False)
    # yT: y_d[r, k, j] = y[r, j*128+k]
    y_d = nc.declare_dram_parameter("y", [rows, M, NBLK], BF16, isOutput=True)

    # --- SBUF tensors ---
    w_s = nc.alloc_sbuf_tensor("w_s", [M, 2, rows, M], BF16).ap()
    xrow = [
        nc.alloc_sbuf_tensor(f"xrow{r}", [M, NBLK + 1], BF16).ap()
        for r in range(rows)
    ]
    yrow = [
        nc.alloc_sbuf_tensor(f"yrow{r}", [M, NBLK], BF16).ap() for r in range(rows)
    ]

    # --- PSUM: 8-deep rotation of fp32 banks for the matmul accumulator,
    # so the PE runs up to 7 chunks (a full row) ahead of the ACT/DVE
    # evacuations and never stalls on their semaphore round-trips ---
    NPS = 8
    y_ps = [
        nc.alloc_psum_tensor(f"yps{i}", [M, CHUNK], F32).ap() for i in range(NPS)
    ]

    with (
        nc.Block() as block,
        nc.semaphore("s_w") as s_w,
        nc.semaphore("s_x0") as s_x0,
        nc.semaphore("s_x1") as s_x1,
        nc.semaphore("s_x2") as s_x2,
        nc.semaphore("s_x3") as s_x3,
        nc.semaphore("s_x4") as s_x4,
        nc.semaphore("s_x5") as s_x5,
        nc.semaphore("s_x6") as s_x6,
        nc.semaphore("s_x7") as s_x7,
        nc.semaphore("s_pe") as s_pe,
        nc.semaphore("s_ya") as s_ya,
        nc.semaphore("s_yd") as s_yd,
        nc.semaphore("s_st") as s_st,
    ):
        s_x = [s_x0, s_x1, s_x2, s_x3, s_x4, s_x5, s_x6, s_x7][:rows]

        @block.tensor
        def _(pe: bass.BassEngine):
            W = _Waiter(pe)
            W.need(s_w, 16)
            gch = 0
            for r in range(rows):
                W.need(s_x[r], 16)
                for ch in range(NCH):
                    pp = gch % NPS
                    # WAR: y_ps[pp] must be evacuated from chunk gch-NPS
                    if gch >= NPS:
                        if gch % 2 == 0:
                            W.need(s_ya, (gch - NPS) // 2 + 1)
                        else:
                            W.need(s_yd, (gch - NPS - 1) // 2 + 1)
                    nc.tensor.matmul(
                        y_ps[pp],
                        lhsT=w_s[:, 0, r],
                        rhs=xrow[r][:, 1 + ch * CHUNK : 1 + (ch + 1) * CHUNK],
                        start=True,
                        stop=False,
                    )
                    nc.tensor.matmul(
                        y_ps[pp],
                        lhsT=w_s[:, 1, r],
                        rhs=xrow[r][:, ch * CHUNK : ch * CHUNK + CHUNK],
                        start=False,
                        stop=True,
                    ).then_inc(s_pe, 1)
                    gch += 1

        @block.scalar
        def _(a: bass.BassEngine):
            # ACT: loads w on its HWDGE queue first (this also arms the queue
            # early - a queue's first DMA pays ~8.5us), then evacuates even
            # chunks (PSUM banks 0/2); stores for the first half of the rows
            # also go on this queue so they overlap the SP-queue loads
            W = _Waiter(a)
            a.dma_start(
                out=w_s, in_=w_d.rearrange("k (a r i) -> k a r i", a=2, r=rows)
            ).then_inc(s_w, 16)
            for gch in range(0, ntot, 2):
                r, ch = divmod(gch, NCH)
                W.need(s_pe, gch + 1)
                a.copy(
                    out=yrow[r][:, ch * CHUNK : (ch + 1) * CHUNK],
                    in_=y_ps[gch % NPS],
                ).then_inc(s_ya, 1)
                if ch == NCH - 2 and r < rows // 2:
                    # row r fully evacuated once DVE finishes chunk r*NCH+7
                    W.need(s_yd, 4 * r + 4)
                    a.dma_start(out=y_d[r], in_=yrow[r]).then_inc(s_st, 16)


        @block.vector
        def _(v: bass.BassEngine):
            # DVE evacuates odd chunks (PSUM banks 1/3)
            W = _Waiter(v)
            for gch in range(1, ntot, 2):
                r, ch = divmod(gch, NCH)
                W.need(s_pe, gch + 1)
                v.tensor_copy(
                    out=yrow[r][:, ch * CHUNK : (ch + 1) * CHUNK],
                    in_=y_ps[gch % NPS],
                ).then_inc(s_yd, 1)

        @block.sync
        def _(sp: bass.BassEngine):
            W = _Waiter(sp)
            # loads via the SP HWDGE queue: descriptor generation happens in
            # hardware, so the 8 row loads stream back-to-back at full rate
            for r in range(rows):
                sp.dma_start(out=xrow[r], in_=x_d[r]).then_inc(s_x[r], 16)
            # stores for the second half of the rows share the SP queue:
            # they enqueue strictly after the loads
            for r in range(rows // 2, rows):
                W.need(s_ya, 4 * r + 4)
                W.need(s_yd, 4 * r + 4)
                sp.dma_start(out=y_d[r], in_=yrow[r]).then_inc(s_st, 16)
            W.need(s_st, 16 * rows)

    return nc


def _get_nc() -> bass.Bass:
    if "nc" not in _CACHED:
        _CACHED["nc"] = _build_bass()
    return _CACHED["nc"]


def run(x, b, a, trace=False, **spmd_kwargs):
    """Shard inputs, run the Bass kernel on 8 cores, gather full output."""
    x = np.asarray(x, dtype=np.float32)
    b = np.asarray(b, dtype=np.float32)
    a = np.asarray(a, dtype=np.float32)
    assert x.shape == (BATCH, T), x.shape
    h = _impulse_response(b, a, NH)
    w1, w2 = _toeplitz_weights(h)  # [BATCH, M(k), M(i)] each, f64
    # w_host[b, k, a, i] -> flatten per-row later
    w = np.stack([w1, w2], axis=2)  # [BATCH, M(k), 2, M(i)]
    w = np.ascontiguousarray(w.transpose(0, 2, 1, 3))  # [BATCH, 2, k, i]

    # host-side transpose: xT[b, k, j] = x[b, j*128+k], with zero col 0
    xt = np.zeros((BATCH, M, NBLK + 1), dtype=NPBF16)
    xt[:, :, 1:] = (
        x.reshape(BATCH, NBLK, M).transpose(0, 2, 1).astype(NPBF16)
    )

    in_maps = []
    for c in range(NCORES):
        rs = slice(c * R, (c + 1) * R)
        # w_core[k, a, r, i] layout flattened to [M, 2*R*M]
        wc = np.ascontiguousarray(
            w[rs].transpose(2, 1, 0, 3).reshape(M, 2 * R * M).astype(NPBF16)
        )
        in_maps.append({"x": np.ascontiguousarray(xt[rs]), "w": wc})
    nc = _get_nc()
    out = run_bass_kernel_spmd(
        nc, in_maps, list(range(NCORES)), trace=trace, **spmd_kwargs
    )
    yt = np.concatenate(
        [np.asarray(out.results[c]["y"]) for c in range(NCORES)], axis=0
    )  # [BATCH, M, NBLK] bf16
    y = yt.transpose(0, 2, 1).reshape(BATCH, T).astype(np.float32)
    return y, out


def kernel(x, b, a):
    y, _ = run(x, b, a)
    return y


# revision 44
# speedup vs baseline: 1.1468x; 1.0037x over previous
"""Biquad IIR filter (direct-form-II-transposed) on 8 Trainium2 NeuronCores.

Strategy
--------
The biquad is stable (|poles| <= ~0.72 for the spec's coefficient
distribution), so its impulse response decays below bf16 resolution well
within 128 taps.  The sequential IIR scan becomes a 128-tap FIR
convolution computed as a block-Toeplitz matmul with blocks of M=128:

    yT[:, j] = A1 @ xT[:, j] + A2 @ xT[:, j-1]
    A1[i,k] = h[i-k]        (lower triangular, current block)
    A2[i,k] = h[128+i-k]    (strict upper triangular, previous block tail)

where xT[k, j] = x[j*128 + k].  The host pre-transposes x into xT (and
un-transposes y afterwards), so the device never transposes anything:
the tensor engine runs exactly two bf16 matmuls (1 cycle/row) per
512-block chunk, with the Toeplitz factors as stationary operands and
xT as the moving operand; the A2 term is the same matmul with the
moving operand shifted one block-column (a zero/carry column baked into
xT col 0 handles the row start).  All tensors are bf16 (inputs rounded
on host, outputs rounded on device and upcast on host), which halves
HBM traffic; per-partition DMA runs are 8KB contiguous so DMA moves at
full rate.

Pipeline (raw Bass, one standalone wait per dependency):

    SP  : per-row xT loads on its HWDGE queue, then stores for rows 4-7
          (they enqueue strictly after the loads, so loads keep the full
          DMA bandwidth while they gate compute)
    ACT : w load first (arms its HWDGE queue early - a queue's first DMA
          pays ~8.5us), evacuates even chunks (PSUM f32 -> SBUF bf16),
          and stores rows 0-3 so stores overlap the load phase
    PE  : per chunk, A1/A2 matmuls into an 8-deep PSUM bank rotation
    DVE : evacuates odd chunks

The DMA engines process descriptors serially at ~26.5GB/s each (~424GB/s
aggregate), so the kernel is DMA-bound end to end: ~9.5us fixed NEFF/
queue-arming head + ~17.3MB of traffic + store tail.

Sharding: data-parallel over the batch axis - 64 rows / 8 cores = 8
rows per core; filters are per-row so there is no cross-core traffic.
"""

import sys

import numpy as np

if "/opt/trn_rl_repo" not in sys.path:
    sys.path.insert(0, "/opt/trn_rl_repo")

import ml_dtypes

import concourse.bass as bass
import concourse.mybir as mybir
from concourse.bass_utils import run_bass_kernel_spmd

BATCH = 64
T = 524288
NCORES = 8
R = BATCH // NCORES  # rows per core
NH = 128  # FIR taps (impulse response length kept)
M = 128  # block length = matmul contraction dim
NBLK = T // M  # 4096 blocks per row
CHUNK = 512  # blocks per chunk = one fp32 PSUM bank
NCH = NBLK // CHUNK  # chunks per row
F32 = mybir.dt.float32
BF16 = mybir.dt.bfloat16
NPBF16 = ml_dtypes.bfloat16

_CACHED = {}


def _impulse_response(b: np.ndarray, a: np.ndarray, n: int) -> np.ndarray:
    """First n samples of the biquad impulse response, computed in f64."""
    nb = b.astype(np.float64)
    na = a.astype(np.float64)
    b0, b1, b2 = nb[:, 0], nb[:, 1], nb[:, 2]
    a1, a2 = na[:, 0], na[:, 1]
    rows = b.shape[0]
    h = np.zeros((rows, n), dtype=np.float64)
    z1 = np.zeros(rows, dtype=np.float64)
    z2 = np.zeros(rows, dtype=np.float64)
    for t in range(n):
        v0 = 1.0 if t == 0 else 0.0
        v1 = b0 * v0 + z1
        nz1 = b1 * v0 - a1 * v1 + z2
        nz2 = b2 * v0 - a2 * v1
        h[:, t] = v1
        z1, z2 = nz1, nz2
    return h


def _toeplitz_weights(h: np.ndarray) -> tuple[np.ndarray, np.ndarray]:
    """Build per-row stationary matmul operands W1T/W2T, each [rows,128,128].

    W1T[r, k, i] = h[r, i-k]      for i >= k   (A1 transposed)
    W2T[r, k, i] = h[r, 128+i-k]  for k >  i   (A2 transposed)
    """
    rows = h.shape[0]
    i = np.arange(M)[None, :]  # output sample within block
    k = np.arange(M)[:, None]  # input sample within block
    d1 = i - k
    w1 = np.zeros((rows, M, M), dtype=np.float64)
    mask1 = d1 >= 0
    w1[:, mask1] = h[:, d1[mask1]]
    d2 = M + i - k
    w2 = np.zeros((rows, M, M), dtype=np.float64)
    mask2 = d2 <= NH - 1
    w2[:, mask2] = h[:, d2[mask2]]
    return w1, w2


class _Waiter:
    """Emit a standalone wait_ge only when the target value increases."""

    def __init__(self, eng):
        self.eng = eng
        self.seen = {}

    def need(self, sem, val):
        if val <= 0:
            return
        if self.seen.get(sem.name, -1) >= val:
            return
        self.seen[sem.name] = val
        self.eng.wait_ge(sem, val)


def _build_bass(rows: int = R) -> bass.Bass:
    ntot = rows * NCH

    nc = bass.Bass(trn_type="TRN2")
    # xT with a leading zero/carry column: x_d[r, k, 1+j] = x[r, j*128+k]
    x_d = nc.declare_dram_parameter("x", [rows, M, NBLK + 1], BF16, isOutput=False)
    # w_d[k, a, r, i] = WaT[r, k, i]
    w_d = nc.declare_dram_parameter("w", [M, 2 * rows * M], BF16, isOutput=False)
    # yT: y_d[r, k, j] = y[r, j*128+k]
    y_d = nc.declare_dram_parameter("y", [rows, M, NBLK], BF16, isOutput=True)

    # --- SBUF tensors ---
    w_s = nc.alloc_sbuf_tensor("w_s", [M, 2, rows, M], BF16).ap()
    xrow = [
        nc.alloc_sbuf_tensor(f"xrow{r}", [M, NBLK + 1], BF16).ap()
        for r in range(rows)
    ]
    yrow = [
        nc.alloc_sbuf_tensor(f"yrow{r}", [M, NBLK], BF16).ap() for r in range(rows)
    ]

    # --- PSUM: 8-deep rotation of fp32 banks for the matmul accumulator,
    # so the PE runs up to 7 chunks (a full row) ahead of the ACT/DVE
    # evacuations and never stalls on their semaphore round-trips ---
    NPS = 8
    y_ps = [
        nc.alloc_psum_tensor(f"yps{i}", [M, CHUNK], F32).ap() for i in range(NPS)
    ]

    with (
        nc.Block() as block,
        nc.semaphore("s_w") as s_w,
        nc.semaphore("s_x0") as s_x0,
        nc.semaphore("s_x1") as s_x1,
        nc.semaphore("s_x2") as s_x2,
        nc.semaphore("s_x3") as s_x3,
        nc.semaphore("s_x4") as s_x4,
        nc.semaphore("s_x5") as s_x5,
        nc.semaphore("s_x6") as s_x6,
        nc.semaphore("s_x7") as s_x7,
        nc.semaphore("s_pe") as s_pe,
        nc.semaphore("s_ya") as s_ya,
        nc.semaphore("s_yd") as s_yd,
        nc.semaphore("s_st") as s_st,
    ):
        s_x = [s_x0, s_x1, s_x2, s_x3, s_x4, s_x5, s_x6, s_x7][:rows]

        @block.tensor
        def _(pe: bass.BassEngine):
            W = _Waiter(pe)
            W.need(s_w, 16)
            gch = 0
            for r in range(rows):
                W.need(s_x[r], 16)
                for ch in range(NCH):
                    pp = gch % NPS
                    # WAR: y_ps[pp] must be evacuated from chunk gch-NPS
                    if gch >= NPS:
                        if gch % 2 == 0:
                            W.need(s_ya, (gch - NPS) // 2 + 1)
                        else:
                            W.need(s_yd, (gch - NPS - 1) // 2 + 1)
                    nc.tensor.matmul(
                        y_ps[pp],
                        lhsT=w_s[:, 0, r],
                        rhs=xrow[r][:, 1 + ch * CHUNK : 1 + (ch + 1) * CHUNK],
                        start=True,
                        stop=False,
                    )
                    nc.tensor.matmul(
                        y_ps[pp],
                        lhsT=w_s[:, 1, r],
                        rhs=xrow[r][:, ch * CHUNK : ch * CHUNK + CHUNK],
                        start=False,
                        stop=True,
                    ).then_inc(s_pe, 1)
                    gch += 1

        @block.scalar
        def _(a: bass.BassEngine):
            # ACT: loads w on its HWDGE queue first (this also arms the queue
            # early - a queue's first DMA pays ~8.5us), then evacuates even
            # chunks (PSUM banks 0/2); stores for the first half of the rows
            # also go on this queue so they overlap the SP-queue loads
            W = _Waiter(a)
            a.dma_start(
                out=w_s, in_=w_d.rearrange("k (a r i) -> k a r i", a=2, r=rows)
            ).then_inc(s_w, 16)
            for gch in range(0, ntot, 2):
                r, ch = divmod(gch, NCH)
                W.need(s_pe, gch + 1)
                a.copy(
                    out=yrow[r][:, ch * CHUNK : (ch + 1) * CHUNK],
                    in_=y_ps[gch % NPS],
                ).then_inc(s_ya, 1)
                if ch == NCH - 2 and r < rows // 2:
                    # row r fully evacuated once DVE finishes chunk r*NCH+7
                    W.need(s_yd, 4 * r + 4)
                    a.dma_start(out=y_d[r], in_=yrow[r]).then_inc(s_st, 16)


        @block.vector
        def _(v: bass.BassEngine):
            # DVE evacuates odd chunks (PSUM banks 1/3)
            W = _Waiter(v)
            for gch in range(1, ntot, 2):
                r, ch = divmod(gch, NCH)
                W.need(s_pe, gch + 1)
                v.tensor_copy(
                    out=yrow[r][:, ch * CHUNK : (ch + 1) * CHUNK],
                    in_=y_ps[gch % NPS],
                ).then_inc(s_yd, 1)

        @block.sync
        def _(sp: bass.BassEngine):
            W = _Waiter(sp)
            # loads via the SP HWDGE queue: descriptor generation happens in
            # hardware, so the 8 row loads stream back-to-back at full rate
            for r in range(rows):
                sp.dma_start(out=xrow[r], in_=x_d[r]).then_inc(s_x[r], 16)
            # stores for the second half of the rows share the SP queue:
            # they enqueue strictly after the loads
            for r in range(rows // 2, rows):
                W.need(s_ya, 4 * r + 4)
                W.need(s_yd, 4 * r + 4)
                sp.dma_start(out=y_d[r], in_=yrow[r]).then_inc(s_st, 16)
            W.need(s_st, 16 * rows)

    return nc


def _get_nc() -> bass.Bass:
    if "nc" not in _CACHED:
        _CACHED["nc"] = _build_bass()
    return _CACHED["nc"]


def run(x, b, a, trace=False, **spmd_kwargs):
    """Shard inputs, run the Bass kernel on 8 cores, gather full output."""
    x = np.asarray(x, dtype=np.float32)
    b = np.asarray(b, dtype=np.float32)
    a = np.asarray(a, dtype=np.float32)
    assert x.shape == (BATCH, T), x.shape
    h = _impulse_response(b, a, NH)
    w1, w2 = _toeplitz_weights(h)  # [BATCH, M(k), M(i)] each, f64
    # w_host[b, k, a, i] -> flatten per-row later
    w = np.stack([w1, w2], axis=2)  # [BATCH, M(k), 2, M(i)]
    w = np.ascontiguousarray(w.transpose(0, 2, 1, 3))  # [BATCH, 2, k, i]

    # host-side transpose: xT[b, k, j] = x[b, j*128+k], with zero col 0
    xt = np.zeros((BATCH, M, NBLK + 1), dtype=NPBF16)
    xt[:, :, 1:] = (
        x.reshape(BATCH, NBLK, M).transpose(0, 2, 1).astype(NPBF16)
    )

    in_maps = []
    for c in range(NCORES):
        rs = slice(c * R, (c + 1) * R)
        # w_core[k, a, r, i] layout flattened to [M, 2*R*M]
        wc = np.ascontiguousarray(
            w[rs].transpose(2, 1, 0, 3).reshape(M, 2 * R * M).astype(NPBF16)
        )
        in_maps.append({"x": np.ascontiguousarray(xt[rs]), "w": wc})
    nc = _get_nc()
    out = run_bass_kernel_spmd(
        nc, in_maps, list(range(NCORES)), trace=trace, **spmd_kwargs
    )
    yt = np.concatenate(
        [np.asarray(out.results[c]["y"]) for c in range(NCORES)], axis=0
    )  # [BATCH, M, NBLK] bf16
    y = yt.transpose(0, 2, 1).reshape(BATCH, T).astype(np.float32)
    return y, out


def kernel(x, b, a):
    y, _ = run(x, b, a)
    return y
